# revision 34
# baseline (speedup 1.0000x reference)
"""Multi-head attention (QKV projection + masked softmax + PV) on 8 TRN2
NeuronCores.

Sharding: data-parallel over batch (B=2 -> 2 groups of 4 cores), tensor
parallel over heads (16 heads -> 4 heads per core). Each core computes full
F x T attention for its 4 heads.

Per-core schedule:
  - S^T = K^T x Q^T with K=64 contraction, row-tiled: the two heads of a
    128-partition pair run as concurrent 64x128 PE tiles (T0 rows 0-63,
    T8 rows 64-127) -> 2x throughput on the score matmuls.
  - exp on ScalarE per t-tile over both heads at once ([128, 2, FB] PSUM
    group); mask multiply on DVE (2x bf16 mode); PV lags LAG=2 t-tiles.
  - softmax sums via a leading ones column in the V stationary (M=65 PV,
    sums land on ctx partition 0), inverted in place by the fast custom-DVE
    reciprocal (only correct at base partition 0), broadcast down the output
    partitions via an f16 selector-row matmul; ctx evacuation and K-bias
    adds run on ScalarE to relieve the saturated DVE.
  - minimal prologue (just KT/QT for the first head pair); every other
    projection piece is interleaved into the attention t-loop with
    DMA-arrival/consumption deadlines so the PE never idles long (HAM).
  - bulk loads ride the HW-DGE (sync) queue in deadline order (the SW-DGE
    path is ~3x slower and each dma_start costs ~0.6us of queue issue
    time); output stores ride sync too, prefetches ride gpsimd.
  - 8 dummy matmuls at t=0 warm the PE clock gate before real work.
"""

import os
import sys

for _p in ("/opt/trn_rl_repo",):
    if os.path.isdir(_p) and _p not in sys.path:
        sys.path.insert(0, _p)

import numpy as np
import ml_dtypes

import concourse.tile as tile
from concourse import bacc, mybir
from concourse.bass_utils import run_bass_kernel_spmd

B, F, T, D, N, H = 2, 2048, 2048, 1024, 16, 64
NCORES = 8
HPC = N // (NCORES // B)  # heads per core = 4
NG = HPC // 2             # 128-partition head pairs = 2
FB = 512                  # f-block (psum bank width in fp32)
NJ = F // FB              # 4
NT = T // 128             # 16 t-tiles
NK = D // 128             # 8 contraction tiles
HP1 = H + 1               # head V columns incl. the ones column
LAG = 2                   # PV runs LAG t-tiles behind S/exp

F32 = mybir.dt.float32
F16 = mybir.dt.float16
BF16 = mybir.dt.bfloat16

# projection pieces interleaved into the attention stream, keyed (j, g) ->
# {t_step: [piece...]}; deadlines: KT(tb,g') before S(j0,g',4tb); V(ti)
# before PV consumes it (step ti+LAG); QT(j+1) before j+1 starts.
KV_SCHED = {
    (0, 0): {1: [("v", 0)], 2: [("v", 1)], 3: [("v", 2), ("k", 1, 0)],
             4: [("v", 3)], 5: [("v", 4), ("k", 0, 1)],
             6: [("v", 5), ("k", 2, 0)], 7: [("v", 6)], 8: [("v", 7)],
             9: [("v", 8)], 10: [("v", 9), ("k", 3, 0)], 11: [("v", 10)],
             12: [("v", 11), ("q", 0, 1)], 13: [("v", 12)], 14: [("v", 13)],
             15: [("v", 14), ("v", 15)]},
    (0, 1): {0: [("k", 1, 1)], 1: [("k", 2, 1)], 2: [("k", 3, 1)]},
}


def _emit_k_piece(nc, ps1, kv, KT, bias_sb, tb, g):
    """KT[:, g, tb*FB:(tb+1)*FB] for head pair g (even head rows 0-63, odd
    head rows 64-127)."""
    toT_sb, wk_sb = kv["toT_sb"], kv["wk_sb"]
    ps_qk = ps1.tile([128, FB], F32, tag="qk", name="ps_k")
    for k in range(NK):
        nc.tensor.matmul(
            ps_qk[:],
            wk_sb[:, k, g * 128:(g + 1) * 128],
            toT_sb[:, k, tb * FB:(tb + 1) * FB],
            start=(k == 0),
            stop=(k == NK - 1),
        )
    nc.scalar.add(
        KT[0:64, g, tb * FB:(tb + 1) * FB],
        ps_qk[0:64, :],
        bias_sb[0:64, NG + g:NG + g + 1],
    )
    nc.scalar.add(
        KT[64:128, g, tb * FB:(tb + 1) * FB],
        ps_qk[64:128, :],
        bias_sb[64:128, NG + g:NG + g + 1],
    )


def _emit_v_piece(nc, ps1, kv, Vsb, bv_sb, vones_sb, ti):
    """V[t-tile ti] for all 4 heads, interleaved with the ones columns."""
    toT_sb, wv_sb = kv["toT_sb"], kv["wv_sb"]
    ps_v = ps1.tile([128, HPC * H], F32, tag="qk", name="ps_v")
    for k in range(NK):
        nc.tensor.matmul(
            ps_v[:],
            toT_sb[:, k, ti * 128:(ti + 1) * 128],
            wv_sb[:, k, :],
            start=(k == 0),
            stop=False,
        )
    nc.tensor.matmul(ps_v[:], vones_sb[:], bv_sb[:], start=False, stop=True)
    # one strided cast per t-tile: [128, 4, 64] -> [128, 4, 65][:, :, 0:64]
    dst = Vsb[:, ti, :].rearrange("p (n h1) -> p n h1", n=HPC)
    src = ps_v[:].rearrange("p (n h) -> p n h", n=HPC)
    nc.vector.tensor_copy(dst[:, :, 1:HP1], src)


def _emit_qt(nc, ps1, wq_sb, fromT_tile, QT, bias_sb, j, g):
    ps_qk = ps1.tile([128, FB], F32, tag="qk", name="ps_q")
    for k in range(NK):
        nc.tensor.matmul(
            ps_qk[:],
            wq_sb[:, k, g * 128:(g + 1) * 128],
            fromT_tile[:, k, :],
            start=(k == 0),
            stop=(k == NK - 1),
        )
    nc.vector.tensor_scalar_add(
        QT[:, g, j * FB:(j + 1) * FB],
        ps_qk[:],
        bias_sb[:, g:g + 1],
    )


def _program():
    nc = bacc.Bacc(None, target_bir_lowering=False)
    fromT = nc.declare_dram_parameter("fromT", [D, F], BF16, isOutput=False)
    toT = nc.declare_dram_parameter("toT", [D, T], BF16, isOutput=False)
    maskT = nc.declare_dram_parameter("maskT", [T, F], BF16, isOutput=False)
    wq = nc.declare_dram_parameter("wq", [D, HPC * H], BF16, isOutput=False)
    wk = nc.declare_dram_parameter("wk", [D, HPC * H], BF16, isOutput=False)
    wv = nc.declare_dram_parameter("wv", [D, HPC * H], BF16, isOutput=False)
    bqk = nc.declare_dram_parameter("bqk", [128, 2 * NG], F32, isOutput=False)
    # bv padded to K=128 (row 0 = bv, rest zero) for a mode-switch-free matmul
    bv_pad = nc.declare_dram_parameter("bv_pad", [128, HPC * H], BF16, isOutput=False)
    # all-ones row 0 (rest zero): stationary operand of the bv matmul
    vones = nc.declare_dram_parameter("vones", [128, 128], BF16, isOutput=False)
    # broadcast selector: sel64[k, m] = (k == 0); as lhsT it replicates the
    # reciprocal row (shadow partition 0) down all 128 output partitions
    sel64 = nc.declare_dram_parameter("sel64", [128, 128], F16, isOutput=False)
    out_ctx = nc.declare_dram_parameter("out_ctx", [HPC, H, F], F32, isOutput=True)

    fromT_re = fromT[:].rearrange("(k p) f -> p k f", p=128)
    toT_re = toT[:].rearrange("(k p) t -> p k t", p=128)
    maskT_re = maskT[:].rearrange("(a p) f -> p a f", p=128)

    with tile.TileContext(nc) as tc:
        with (
            tc.tile_pool(name="persist", bufs=1) as persist,
            tc.tile_pool(name="p1", bufs=1) as p1,
            tc.tile_pool(name="pfrom", bufs=2) as pfrom,
            tc.tile_pool(name="p2", bufs=2) as p2,
            tc.tile_pool(name="p2e", bufs=LAG + 4) as p2e,
            tc.tile_pool(name="p2s", bufs=3) as p2s,
            tc.tile_pool(name="p2r", bufs=4) as p2r,
            tc.tile_pool(name="ps1", bufs=2, space="PSUM") as ps1,
            tc.tile_pool(name="ps_s", bufs=2, space="PSUM") as ps_s,
            tc.tile_pool(name="ps_c", bufs=2, space="PSUM") as ps_c,
        ):
            QT = persist.tile([128, NG, F], BF16)   # [h-in-pair, g, f]
            KT = persist.tile([128, NG, T], BF16)   # even head rows 0-63, odd 64-127
            Vsb = persist.tile([128, NT, HPC * HP1], BF16)
            bias_sb = persist.tile([128, 2 * NG], F32)
            bv_sb = persist.tile([128, HPC * H], BF16)
            vones_sb = persist.tile([128, 128], BF16)
            sel64_sb = persist.tile([128, 128], F16)
            # recip shadow: per head nn, row 64 of rsh[:, nn, :] holds 1/sums
            # in f16; all other rows stay zero so the selector matmul is clean
            rsh = persist.tile([128, HPC, FB], F16)
            warm_w = persist.tile([128, 128], BF16)
            warm_m = persist.tile([128, FB], BF16)

            # ---- t=0: warm the ACT table + the PE clock gate (no DMA deps)
            nc.vector.memset(warm_w[:], 0.0)
            nc.vector.memset(warm_m[:], 0.0)
            nc.vector.memset(rsh[:], 0.0)
            for nl in range(HPC):
                nc.vector.memset(Vsb[:, :, nl * HP1], 1.0)
            act_warm = persist.tile([1, 1], F32)
            nc.scalar.activation(act_warm[:], warm_m[0:1, 0:1],
                                 mybir.ActivationFunctionType.Exp)
            ps_warm_a = ps1.tile([128, FB], F32, tag="qk", name="ps_warm")
            ps_warm_b = ps1.tile([128, FB], F32, tag="qk", name="ps_warm")
            for i in range(12):
                nc.tensor.matmul(ps_warm_a[:] if i % 2 == 0 else ps_warm_b[:],
                                 warm_w[:], warm_m[:],
                                 start=True, stop=True)

            # ---- DMA issue. Everything on the j0 critical path goes on the
            # HW-DGE (sync) queue in deadline order -- the SW-DGE (gpsimd)
            # path only sustains ~1/3 the bandwidth. gpsimd carries the
            # late-deadline fromT prefetches and output stores.
            toT_sb = p1.tile([128, NK, T], BF16)
            wq_sb = p1.tile([128, NK, HPC * H], BF16)
            wk_sb = p1.tile([128, NK, HPC * H], BF16)
            wv_sb = p1.tile([128, NK, HPC * H], BF16)
            nc.sync.dma_start(bias_sb[:], bqk[:])
            for k in range(NK):
                nc.sync.dma_start(wk_sb[:, k, :], wk[k * 128:(k + 1) * 128, :])
                nc.sync.dma_start(toT_sb[:, k, 0:FB], toT_re[:, k, 0:FB])
            fromT_t = {}
            fromT_t[0] = pfrom.tile([128, NK, FB], BF16, tag="fromT", name="fromT")
            for k in range(NK):
                nc.gpsimd.dma_start(wq_sb[:, k, :], wq[k * 128:(k + 1) * 128, :])
            nc.gpsimd.dma_start(bv_sb[:], bv_pad[:])
            nc.gpsimd.dma_start(vones_sb[:], vones[:])
            for k in range(NK):
                nc.gpsimd.dma_start(fromT_t[0][:, k, :], fromT_re[:, k, 0:FB])
            nc.gpsimd.dma_start(sel64_sb[:], sel64[:])
            masks = {}
            masks[0] = p2.tile([128, NT, FB], BF16, tag="mask", name="mask")
            nc.sync.dma_start(wv_sb[:], wv[:].rearrange("(k p) m -> p k m", p=128))
            nc.sync.dma_start(masks[0][:], maskT_re[:, :, 0:FB])
            for tb in range(1, NJ):
                nc.sync.dma_start(
                    toT_sb[:, :, tb * FB:(tb + 1) * FB],
                    toT_re[:, :, tb * FB:(tb + 1) * FB],
                )
            fromT_t[1] = pfrom.tile([128, NK, FB], BF16, tag="fromT", name="fromT")
            nc.gpsimd.dma_start(fromT_t[1][:], fromT_re[:, :, FB:2 * FB])

            kv = dict(toT_sb=toT_sb, wk_sb=wk_sb, wv_sb=wv_sb)

            # ---- minimal prologue: just what S(j0, g0, t0) needs; every
            # other projection piece is interleaved into the t-loop
            _emit_k_piece(nc, ps1, kv, KT, bias_sb, 0, 0)
            _emit_qt(nc, ps1, wq_sb, fromT_t[0], QT, bias_sb, 0, 0)

            # ---- attention ----
            pending_norm = None
            for j in range(NJ):
                mask_j = masks.pop(j)
                if j + 1 < NJ:
                    masks[j + 1] = p2.tile([128, NT, FB], BF16,
                                           tag="mask", name="mask")
                    nc.sync.dma_start(
                        masks[j + 1][:],
                        maskT_re[:, :, (j + 1) * FB:(j + 2) * FB],
                    )
                if j + 2 < NJ:
                    fromT_t[j + 2] = pfrom.tile([128, NK, FB], BF16,
                                                tag="fromT", name="fromT")
                    nc.gpsimd.dma_start(
                        fromT_t[j + 2][:],
                        fromT_re[:, :, (j + 2) * FB:(j + 3) * FB],
                    )
                ctx_all = []
                for g in range(NG):
                    ne, no = 2 * g, 2 * g + 1
                    ps_ctx_e = ps_c.tile([HP1, FB], F32, tag="ctx", name="ctx_e")
                    ps_ctx_o = ps_c.tile([HP1, FB], F32, tag="ctx", name="ctx_o")
                    sched = KV_SCHED.get((j, g), {})
                    pend = []
                    for t in range(NT):
                        for piece in sched.get(t, ()):
                            if piece[0] == "k":
                                _emit_k_piece(nc, ps1, kv, KT, bias_sb,
                                              piece[1], piece[2])
                            elif piece[0] == "q":
                                _emit_qt(nc, ps1, wq_sb, fromT_t[piece[1]],
                                         QT, bias_sb, piece[1], piece[2])
                            else:
                                _emit_v_piece(nc, ps1, kv, Vsb, bv_sb,
                                              vones_sb, piece[1])
                        if g == 1 and j + 1 < NJ and t == 10:
                            _emit_qt(nc, ps1, wq_sb, fromT_t[j + 1], QT,
                                     bias_sb, j + 1, 0)
                        if g == 1 and j + 1 < NJ and t == 13:
                            _emit_qt(nc, ps1, wq_sb, fromT_t[j + 1], QT,
                                     bias_sb, j + 1, 1)
                        # S pair: two concurrent 64x128 row tiles
                        ps_sq = ps_s.tile([128, 2, FB], F32, tag="sq", name="sq")
                        nc.tensor.matmul(
                            ps_sq[:, 0, :],
                            KT[0:64, g, t * 128:(t + 1) * 128],
                            QT[0:64, g, j * FB:(j + 1) * FB],
                            start=True, stop=True,
                        )
                        nc.tensor.matmul(
                            ps_sq[:, 1, :],
                            KT[64:128, g, t * 128:(t + 1) * 128],
                            QT[64:128, g, j * FB:(j + 1) * FB],
                            start=True, stop=True,
                        )
                        ex = p2e.tile([128, 2, FB], BF16, tag="exp", name="exp")
                        nc.scalar.activation(
                            ex[:], ps_sq[:],
                            mybir.ActivationFunctionType.Exp,
                            scale=0.125,
                        )
                        nc.vector.tensor_mul(ex[:, 0, :], ex[:, 0, :],
                                             mask_j[:, t, :])
                        nc.vector.tensor_mul(ex[:, 1, :], ex[:, 1, :],
                                             mask_j[:, t, :])
                        pend.append((t, ex))
                        if pending_norm is not None and g == 0 and t in (0, 2):
                            pending_norm(0 if t == 0 else 1)
                            if t == 2:
                                pending_norm = None
                        if len(pend) > LAG:
                            tp, exp_t = pend.pop(0)
                            nc.tensor.matmul(
                                ps_ctx_e[:], Vsb[:, tp, ne * HP1:(ne + 1) * HP1],
                                exp_t[:, 0, :],
                                start=(tp == 0), stop=(tp == NT - 1),
                            )
                            nc.tensor.matmul(
                                ps_ctx_o[:], Vsb[:, tp, no * HP1:(no + 1) * HP1],
                                exp_t[:, 1, :],
                                start=(tp == 0), stop=(tp == NT - 1),
                            )
                    for tp, exp_t in pend:
                        nc.tensor.matmul(
                            ps_ctx_e[:], Vsb[:, tp, ne * HP1:(ne + 1) * HP1],
                            exp_t[:, 0, :],
                            start=(tp == 0), stop=(tp == NT - 1),
                        )
                        nc.tensor.matmul(
                            ps_ctx_o[:], Vsb[:, tp, no * HP1:(no + 1) * HP1],
                            exp_t[:, 1, :],
                            start=(tp == 0), stop=(tp == NT - 1),
                        )
                    ctx_e = p2s.tile([HP1, FB], F32, tag="ctx_sb",
                                     name="ctx_sb", bufs=6)
                    ctx_o = p2s.tile([HP1, FB], F32, tag="ctx_sb",
                                     name="ctx_sb", bufs=6)
                    nc.scalar.copy(ctx_e[:], ps_ctx_e[:])
                    nc.scalar.copy(ctx_o[:], ps_ctx_o[:])
                    # softmax denominators sit at partition 0 (the ones
                    # column is FIRST in the V stationary); invert in place --
                    # the custom recip op is only correct at base partition 0
                    nc.vector.reciprocal_approx_fast(ctx_e[0:1, :],
                                                     ctx_e[0:1, :])
                    nc.vector.reciprocal_approx_fast(ctx_o[0:1, :],
                                                     ctx_o[0:1, :])
                    nc.vector.tensor_copy(rsh[0:1, ne, :], ctx_e[0:1, :])
                    nc.vector.tensor_copy(rsh[0:1, no, :], ctx_o[0:1, :])
                    ctx_all.append((ne, ctx_e))
                    ctx_all.append((no, ctx_o))

                    def _norm_pair(nn_ctx, j=j):
                        for nn, ctx_sb in nn_ctx:
                            ps_bc = ps1.tile([128, FB], F32, tag="qk",
                                             name="ps_bc")
                            nc.tensor.matmul(
                                ps_bc[:],
                                sel64_sb[0:HP1, :],
                                rsh[0:HP1, nn, :],
                                start=True, stop=True,
                            )
                            out_sb = p2s.tile([HP1, FB], F32, tag="out")
                            nc.vector.tensor_mul(
                                out_sb[:], ctx_sb[:], ps_bc[0:HP1, :]
                            )
                            nc.sync.dma_start(
                                out_ctx[nn, :, j * FB:(j + 1) * FB],
                                out_sb[1:HP1, :],
                            )
                    if j == NJ - 1:
                        _norm_pair(ctx_all[-2:])
                if j < NJ - 1:
                    def pending_norm(step, ctx_all=list(ctx_all), j=j):
                        _ctx = ctx_all[0:2] if step == 0 else ctx_all[2:4]
                        for nn, ctx_sb in _ctx:
                            ps_bc = ps1.tile([128, FB], F32, tag="qk",
                                             name="ps_bc")
                            nc.tensor.matmul(
                                ps_bc[:],
                                sel64_sb[0:HP1, :],
                                rsh[0:HP1, nn, :],
                                start=True, stop=True,
                            )
                            out_sb = p2s.tile([HP1, FB], F32, tag="out")
                            nc.vector.tensor_mul(
                                out_sb[:], ctx_sb[:], ps_bc[0:HP1, :]
                            )
                            nc.sync.dma_start(
                                out_ctx[nn, :, j * FB:(j + 1) * FB],
                                out_sb[1:HP1, :],
                            )

    nc.compile()
    return nc


_compiled = None


def _get_compiled():
    global _compiled
    if _compiled is None:
        _compiled = _program()
    return _compiled


def make_in_maps(from_tensor, to_tensor, attention_mask, wq, bq, wk, bk, wv, bv):
    bf = ml_dtypes.bfloat16
    from_tensor = np.asarray(from_tensor, dtype=np.float32)
    to_tensor = np.asarray(to_tensor, dtype=np.float32)
    attention_mask = np.asarray(attention_mask)
    wq = np.asarray(wq, dtype=np.float32)
    wk = np.asarray(wk, dtype=np.float32)
    wv = np.asarray(wv, dtype=np.float32)
    bq = np.asarray(bq, dtype=np.float32)
    bk = np.asarray(bk, dtype=np.float32)
    bv = np.asarray(bv, dtype=np.float32)

    fromT_b = [np.ascontiguousarray(from_tensor[b].T).astype(bf) for b in range(B)]
    toT_b = [np.ascontiguousarray(to_tensor[b].T).astype(bf) for b in range(B)]
    maskT_b = [attention_mask[b].T.astype(bf) for b in range(B)]
    vones_arr = np.zeros((128, 128), dtype=bf)
    vones_arr[0, :] = 1.0
    sel64_arr = np.zeros((128, 128), dtype=np.float16)
    sel64_arr[0, :] = 1.0

    in_maps = []
    for c in range(NCORES):
        b, hb = divmod(c, NCORES // B)
        hs = hb * HPC
        bq_dev = bq[hs:hs + HPC].reshape(NG, 128).T
        bk_dev = bk[hs:hs + HPC].reshape(NG, 128).T
        bv_pad = np.zeros((128, HPC * H), dtype=bf)
        bv_pad[0, :] = bv[hs:hs + HPC].reshape(HPC * H)
        in_maps.append(
            dict(
                fromT=fromT_b[b],
                toT=toT_b[b],
                maskT=maskT_b[b],
                wq=wq[:, hs:hs + HPC, :].reshape(D, HPC * H).astype(bf),
                wk=wk[:, hs:hs + HPC, :].reshape(D, HPC * H).astype(bf),
                wv=wv[:, hs:hs + HPC, :].reshape(D, HPC * H).astype(bf),
                bqk=np.ascontiguousarray(
                    np.concatenate([bq_dev, bk_dev], axis=1), dtype=np.float32
                ),
                bv_pad=bv_pad,
                vones=vones_arr,
                sel64=sel64_arr,
            )
        )
    return in_maps


def gather_output(results):
    out = np.empty((B, F, N, H), dtype=np.float32)
    for c in range(NCORES):
        b, hb = divmod(c, NCORES // B)
        hs = hb * HPC
        ctx = results[c]["out_ctx"]  # [HPC, H, F]
        out[b, :, hs:hs + HPC, :] = ctx.transpose(2, 0, 1)
    return out


def run_sharded(inputs, **run_kwargs):
    """Run the SPMD kernel; returns (output, BassKernelResults)."""
    nc = _get_compiled()
    in_maps = make_in_maps(**inputs)
    res = run_bass_kernel_spmd(nc, in_maps, list(range(NCORES)), **run_kwargs)
    return gather_output(res.results), res


def kernel(**inputs):
    out, _ = run_sharded(inputs)
    return out
